# revision 6
# baseline (speedup 1.0000x reference)
"""Trainium2 Bass kernel for CorrectedPartialCharges.

out[i] = pc[i] + (total_charge[g] - seg_sum[g]) / n_atoms[g],  g = i // 256

Sharding: graphs are data-parallel across the 8 cores (4096 graphs /
1,048,576 atoms per core); segment sums and the gather-broadcast stay
device-local. On each core, partition p owns 32 contiguous graphs.

Wire format is bf16 (the 2e-2 rel-err budget allows it): node charges are
rounded to bf16 on the host, halving HBM traffic both ways; the device
accumulates segment sums in fp32. total_charge is pre-divided by 256 on
the host so the leftover is one fused scalar_tensor_tensor op.

Per [128, w] tile (k = w/256 graphs per partition):
  - k tensor_scalar(mult 1.0, accum_out) ops: identity rewrite of x that
    emits the per-graph segment sum as the instruction accumulator
    (4x-capable op, vs the 1x tensor_reduce).
  - left = (seg * -1/256) + tc/256  (one scalar_tensor_tensor)
  - k per-graph tensor_scalar adds, split across Vector/GpSimd/Scalar
    engines so no single engine serializes the kernel.
DMA: loads/stores alternate between the two HWDGE rings (sync/scalar),
tc rides SWDGE (gpsimd), the last store is split across both rings to
shorten the tail.
"""

import ml_dtypes
import numpy as np

import concourse.bacc as bacc
import concourse.bass as bass  # noqa: F401
import concourse.mybir as mybir
import concourse.tile as tile
from concourse.bass_utils import run_bass_kernel_spmd

N_CORES = 8
ATOMS_PER_GRAPH = 256
N_GRAPHS = 32768
N_ATOMS = N_GRAPHS * ATOMS_PER_GRAPH
P = 128

G_PER_CORE = N_GRAPHS // N_CORES          # 4096 graphs per core
A_PER_CORE = G_PER_CORE * ATOMS_PER_GRAPH  # 1,048,576 atoms per core

# Knobs read by test.py when experimenting.
NT = 4                    # tiles per core along the free dim
RED_MODE = "ts_accum"     # "ts_accum" | "halve" | "reduce"
# engine per j-index for the per-graph adds (cycled if k > len)
ADD_ENGINES = ("vector", "vector", "gpsimd", "gpsimd", "scalar", "scalar",
               "vector", "vector")
SPLIT_LAST_STORE = True
TC_ON_GPSIMD = True

_TRACE = False
_TRACE_KWARGS = {}


def _build(nt=None, red_mode=None, add_engines=None, split_last=None,
           tc_gpsimd=None):
    nt = NT if nt is None else nt
    red_mode = RED_MODE if red_mode is None else red_mode
    add_engines = ADD_ENGINES if add_engines is None else add_engines
    split_last = SPLIT_LAST_STORE if split_last is None else split_last
    tc_gpsimd = TC_ON_GPSIMD if tc_gpsimd is None else tc_gpsimd

    ap_free = A_PER_CORE // P     # 8192 atoms per partition
    gp = G_PER_CORE // P          # 32 graphs per partition
    w = ap_free // nt             # atoms per partition per tile
    k = w // ATOMS_PER_GRAPH      # graphs per partition per tile
    half = ATOMS_PER_GRAPH // 2
    assert ap_free % nt == 0 and w % ATOMS_PER_GRAPH == 0

    nc = bacc.Bacc(None, target_bir_lowering=False)

    pc = nc.dram_tensor("pc", [A_PER_CORE], mybir.dt.bfloat16, kind="ExternalInput")
    # total_charge / 256, fp32
    tcs = nc.dram_tensor("tcs", [G_PER_CORE], mybir.dt.float32, kind="ExternalInput")
    out = nc.dram_tensor("out", [A_PER_CORE], mybir.dt.bfloat16, kind="ExternalOutput")

    pc_v = pc[:].rearrange("(p n) -> p n", p=P)
    out_v = out[:].rearrange("(p n) -> p n", p=P)
    tcs_v = tcs[:].rearrange("(p k) -> p k", p=P)

    def load_eng(t):
        return nc.sync if t % 2 == 0 else nc.scalar

    def store_eng(t):
        return nc.scalar if t % 2 == 0 else nc.sync

    with tile.TileContext(nc) as tc:
        with (
            tc.tile_pool(name="io", bufs=nt) as io_pool,
            tc.tile_pool(name="half", bufs=2) as half_pool,
            tc.tile_pool(name="small", bufs=2 * nt) as small_pool,
            tc.tile_pool(name="consts", bufs=1) as const_pool,
        ):
            # Queue every input tile load up front, alternating HWDGE rings.
            xs = []
            for t in range(nt):
                x = io_pool.tile([P, w], mybir.dt.bfloat16, tag="x")
                load_eng(t).dma_start(out=x[:], in_=pc_v[:, t * w : (t + 1) * w])
                xs.append(x)

            tc_tile = const_pool.tile([P, gp], mybir.dt.float32, tag="tc")
            (nc.gpsimd if tc_gpsimd else nc.sync).dma_start(
                out=tc_tile[:], in_=tcs_v
            )

            for t in range(nt):
                x = xs[t]
                x3 = x[:].rearrange("p (k a) -> p k a", a=ATOMS_PER_GRAPH)

                seg = small_pool.tile([P, k], mybir.dt.float32, tag="seg")
                if red_mode == "ts_accum":
                    for j in range(k):
                        blk = x[:, j * ATOMS_PER_GRAPH : (j + 1) * ATOMS_PER_GRAPH]
                        nc.vector.tensor_scalar(
                            out=blk,
                            in0=blk,
                            scalar1=1.0,
                            scalar2=0.0,
                            op0=mybir.AluOpType.mult,
                            op1=mybir.AluOpType.add,
                            accum_out=seg[:, j : j + 1],
                        )
                elif red_mode == "halve":
                    u = half_pool.tile([P, k * half], mybir.dt.bfloat16, tag="u")
                    u3 = u[:].rearrange("p (k a) -> p k a", a=half)
                    nc.vector.tensor_add(
                        out=u3, in0=x3[:, :, 0:half], in1=x3[:, :, half:ATOMS_PER_GRAPH]
                    )
                    nc.vector.reduce_sum(out=seg[:], in_=u3, axis=mybir.AxisListType.X)
                else:
                    nc.vector.reduce_sum(out=seg[:], in_=x3, axis=mybir.AxisListType.X)

                # left = (seg * -1/256) + tc/256   (fused)
                left = small_pool.tile([P, k], mybir.dt.float32, tag="left")
                nc.vector.scalar_tensor_tensor(
                    out=left[:],
                    in0=seg[:],
                    scalar=-1.0 / ATOMS_PER_GRAPH,
                    in1=tc_tile[:, t * k : (t + 1) * k],
                    op0=mybir.AluOpType.mult,
                    op1=mybir.AluOpType.add,
                )

                for j in range(k):
                    blk = x[:, j * ATOMS_PER_GRAPH : (j + 1) * ATOMS_PER_GRAPH]
                    eng = add_engines[j % len(add_engines)]
                    if eng == "scalar":
                        nc.scalar.add(out=blk, in_=blk, add=left[:, j : j + 1])
                    else:
                        getattr(nc, eng).tensor_scalar_add(
                            out=blk, in0=blk, scalar1=left[:, j : j + 1]
                        )
                if split_last and t == nt - 1:
                    hw_ = w // 2
                    nc.scalar.dma_start(
                        out=out_v[:, t * w : t * w + hw_], in_=x[:, 0:hw_]
                    )
                    nc.sync.dma_start(
                        out=out_v[:, t * w + hw_ : (t + 1) * w], in_=x[:, hw_:w]
                    )
                else:
                    store_eng(t).dma_start(out=out_v[:, t * w : (t + 1) * w], in_=x[:])

    nc.finalize()
    return nc


_NC_CACHE = {}


def _get_nc():
    key = (NT, RED_MODE, tuple(ADD_ENGINES), SPLIT_LAST_STORE, TC_ON_GPSIMD)
    if key not in _NC_CACHE:
        _NC_CACHE[key] = _build()
    return _NC_CACHE[key]


def _cpu_fallback(pc, total_charge, batch, n_atoms):
    num_segments = n_atoms.shape[0]
    seg = np.bincount(batch, weights=pc.astype(np.float64), minlength=num_segments)
    leftover = (total_charge - seg.astype(np.float32)) / n_atoms.astype(np.float32)
    return (pc + leftover[batch]).astype(np.float32)


def kernel(**inputs) -> np.ndarray:
    pc = np.ascontiguousarray(
        np.asarray(inputs["node_outputs"], dtype=np.float32).reshape(-1)
    )
    total_charge = np.ascontiguousarray(
        np.asarray(inputs["total_charge"], dtype=np.float32).reshape(-1)
    )
    batch = np.asarray(inputs["batch"]).reshape(-1)
    n_atoms = np.ascontiguousarray(np.asarray(inputs["n_atoms"], dtype=np.int32).reshape(-1))

    # The device kernel hardcodes the uniform 256-atoms-per-graph layout the
    # reference generator produces; anything else goes through numpy.
    if (
        pc.shape[0] != N_ATOMS
        or total_charge.shape[0] != N_GRAPHS
        or not np.all(n_atoms == ATOMS_PER_GRAPH)
        or not np.array_equal(
            batch.astype(np.int64),
            np.arange(N_ATOMS, dtype=np.int64) // ATOMS_PER_GRAPH,
        )
    ):
        return _cpu_fallback(pc, total_charge, batch, n_atoms)

    pc_b = pc.astype(ml_dtypes.bfloat16)
    tcs = (total_charge * (1.0 / ATOMS_PER_GRAPH)).astype(np.float32)

    nc = _get_nc()
    in_maps = []
    for c in range(N_CORES):
        in_maps.append(
            {
                "pc": pc_b[c * A_PER_CORE : (c + 1) * A_PER_CORE],
                "tcs": tcs[c * G_PER_CORE : (c + 1) * G_PER_CORE],
            }
        )
    res = run_bass_kernel_spmd(
        nc, in_maps, list(range(N_CORES)), trace=_TRACE, **_TRACE_KWARGS
    )
    out = np.concatenate([r["out"] for r in res.results]).astype(np.float32)
    if _TRACE:
        kernel.last_results = res
    return out


# revision 7
# speedup vs baseline: 2.3671x; 2.3671x over previous
"""Trainium2 Bass kernel for CorrectedPartialCharges.

out[i] = pc[i] + (total_charge[g] - seg_sum[g]) / n_atoms[g],  g = i // 256

Sharding: graphs are data-parallel across the 8 cores (4096 graphs /
1,048,576 atoms per core); segment sums and the gather-broadcast stay
device-local. On each core, partition p owns 32 contiguous graphs.

Wire format is bf16 (the 2e-2 rel-err budget allows it): node charges are
rounded to bf16 on the host, halving HBM traffic both ways; the device
accumulates segment sums in fp32. total_charge is pre-divided by 256 on
the host so the leftover is one fused scalar_tensor_tensor op.

Engine split (per [128, w] tile, k = w/256 graphs per partition):
  - Vector: TT-halving add (2x mode) + 1x reduce -> seg, fused leftover,
    and the per-graph tensor_scalar adds for the "vector" tiles.
  - Scalar/ACT: per-graph bias adds for the "scalar" tiles (whole-tile
    granularity -- the tile dependency tracker serializes cross-engine
    writes to one tile). ACT tiles are loaded first so ACT starts early.
  - GpSimd: only the small total-charge DMA (SWDGE) -- its tensor ops
    have ~4.7us dispatch overhead and are useless here.
DMA: loads/stores alternate between the two HWDGE rings; the last store
is split across both rings to shorten the drain tail.
"""

import ml_dtypes
import numpy as np

import concourse.bacc as bacc
import concourse.bass as bass  # noqa: F401
import concourse.mybir as mybir
import concourse.tile as tile
from concourse.bass_utils import run_bass_kernel_spmd

N_CORES = 8
ATOMS_PER_GRAPH = 256
N_GRAPHS = 32768
N_ATOMS = N_GRAPHS * ATOMS_PER_GRAPH
P = 128

G_PER_CORE = N_GRAPHS // N_CORES          # 4096 graphs per core
A_PER_CORE = G_PER_CORE * ATOMS_PER_GRAPH  # 1,048,576 atoms per core

# Knobs read by test.py when experimenting.
NT = 4                    # tiles per core along the free dim
# processing order; tiles listed first are loaded first
TILE_ORDER = (2, 3, 0, 1)
# add engine per tile index ("vector" or "scalar")
TILE_ADD_ENGINE = {2: "scalar", 3: "scalar", 0: "vector", 1: "vector"}
SPLIT_LAST_STORE = True
NO_PARTITION_ID = True

_TRACE = False
_TRACE_KWARGS = {}


def _build(nt=None, order=None, add_eng=None, split_last=None, no_pid=None):
    nt = NT if nt is None else nt
    order = TILE_ORDER if order is None else order
    add_eng = TILE_ADD_ENGINE if add_eng is None else add_eng
    split_last = SPLIT_LAST_STORE if split_last is None else split_last
    no_pid = NO_PARTITION_ID if no_pid is None else no_pid

    ap_free = A_PER_CORE // P     # 8192 atoms per partition
    gp = G_PER_CORE // P          # 32 graphs per partition
    w = ap_free // nt             # atoms per partition per tile
    k = w // ATOMS_PER_GRAPH      # graphs per partition per tile
    half = ATOMS_PER_GRAPH // 2
    assert ap_free % nt == 0 and w % ATOMS_PER_GRAPH == 0
    assert tuple(sorted(order)) == tuple(range(nt))

    kwargs = {"enable_partition_id": False} if no_pid else {}
    nc = bacc.Bacc(None, target_bir_lowering=False, **kwargs)

    pc = nc.dram_tensor("pc", [A_PER_CORE], mybir.dt.bfloat16, kind="ExternalInput")
    # total_charge / 256, fp32
    tcs = nc.dram_tensor("tcs", [G_PER_CORE], mybir.dt.float32, kind="ExternalInput")
    out = nc.dram_tensor("out", [A_PER_CORE], mybir.dt.bfloat16, kind="ExternalOutput")

    pc_v = pc[:].rearrange("(p n) -> p n", p=P)
    out_v = out[:].rearrange("(p n) -> p n", p=P)
    tcs_v = tcs[:].rearrange("(p k) -> p k", p=P)

    with tile.TileContext(nc) as tc:
        with (
            tc.tile_pool(name="io", bufs=nt) as io_pool,
            tc.tile_pool(name="half", bufs=2) as half_pool,
            tc.tile_pool(name="small", bufs=2 * nt) as small_pool,
            tc.tile_pool(name="consts", bufs=1) as const_pool,
        ):
            # Queue every input tile load up front, alternating HWDGE rings.
            xs = {}
            for i, t in enumerate(order):
                x = io_pool.tile([P, w], mybir.dt.bfloat16, tag="x")
                eng = nc.sync if i % 2 == 0 else nc.scalar
                eng.dma_start(out=x[:], in_=pc_v[:, t * w : (t + 1) * w])
                xs[t] = x

            tc_tile = const_pool.tile([P, gp], mybir.dt.float32, tag="tc")
            nc.gpsimd.dma_start(out=tc_tile[:], in_=tcs_v)

            for i, t in enumerate(order):
                x = xs[t]
                x3 = x[:].rearrange("p (k a) -> p k a", a=ATOMS_PER_GRAPH)

                # seg sums: TT-halving (2x mode) + 1x reduce on the halves
                seg = small_pool.tile([P, k], mybir.dt.float32, tag="seg")
                u = half_pool.tile([P, k * half], mybir.dt.bfloat16, tag="u")
                u3 = u[:].rearrange("p (k a) -> p k a", a=half)
                nc.vector.tensor_add(
                    out=u3, in0=x3[:, :, 0:half], in1=x3[:, :, half:ATOMS_PER_GRAPH]
                )
                nc.vector.reduce_sum(out=seg[:], in_=u3, axis=mybir.AxisListType.X)

                # left = (seg * -1/256) + tc/256   (fused)
                left = small_pool.tile([P, k], mybir.dt.float32, tag="left")
                nc.vector.scalar_tensor_tensor(
                    out=left[:],
                    in0=seg[:],
                    scalar=-1.0 / ATOMS_PER_GRAPH,
                    in1=tc_tile[:, t * k : (t + 1) * k],
                    op0=mybir.AluOpType.mult,
                    op1=mybir.AluOpType.add,
                )

                for j in range(k):
                    blk = x[:, j * ATOMS_PER_GRAPH : (j + 1) * ATOMS_PER_GRAPH]
                    if add_eng[t] == "scalar":
                        nc.scalar.add(out=blk, in_=blk, add=left[:, j : j + 1])
                    else:
                        nc.vector.tensor_scalar_add(
                            out=blk, in0=blk, scalar1=left[:, j : j + 1]
                        )
                if split_last and i == nt - 1:
                    hw_ = w // 2
                    nc.scalar.dma_start(
                        out=out_v[:, t * w : t * w + hw_], in_=x[:, 0:hw_]
                    )
                    nc.sync.dma_start(
                        out=out_v[:, t * w + hw_ : (t + 1) * w], in_=x[:, hw_:w]
                    )
                else:
                    # stores for ACT-add tiles go on sync so they don't queue
                    # behind the ACT add chain; vector tiles use the ACT ring.
                    eng = nc.sync if add_eng[t] == "scalar" else nc.scalar
                    eng.dma_start(out=out_v[:, t * w : (t + 1) * w], in_=x[:])

    nc.finalize()
    return nc


_NC_CACHE = {}


def _get_nc():
    key = (NT, TILE_ORDER, tuple(sorted(TILE_ADD_ENGINE.items())),
           SPLIT_LAST_STORE, NO_PARTITION_ID)
    if key not in _NC_CACHE:
        _NC_CACHE[key] = _build()
    return _NC_CACHE[key]


def _cpu_fallback(pc, total_charge, batch, n_atoms):
    num_segments = n_atoms.shape[0]
    seg = np.bincount(batch, weights=pc.astype(np.float64), minlength=num_segments)
    leftover = (total_charge - seg.astype(np.float32)) / n_atoms.astype(np.float32)
    return (pc + leftover[batch]).astype(np.float32)


def kernel(**inputs) -> np.ndarray:
    pc = np.ascontiguousarray(
        np.asarray(inputs["node_outputs"], dtype=np.float32).reshape(-1)
    )
    total_charge = np.ascontiguousarray(
        np.asarray(inputs["total_charge"], dtype=np.float32).reshape(-1)
    )
    batch = np.asarray(inputs["batch"]).reshape(-1)
    n_atoms = np.ascontiguousarray(np.asarray(inputs["n_atoms"], dtype=np.int32).reshape(-1))

    # The device kernel hardcodes the uniform 256-atoms-per-graph layout the
    # reference generator produces; anything else goes through numpy.
    if (
        pc.shape[0] != N_ATOMS
        or total_charge.shape[0] != N_GRAPHS
        or not np.all(n_atoms == ATOMS_PER_GRAPH)
        or not np.array_equal(
            batch.astype(np.int64),
            np.arange(N_ATOMS, dtype=np.int64) // ATOMS_PER_GRAPH,
        )
    ):
        return _cpu_fallback(pc, total_charge, batch, n_atoms)

    pc_b = pc.astype(ml_dtypes.bfloat16)
    tcs = (total_charge * (1.0 / ATOMS_PER_GRAPH)).astype(np.float32)

    nc = _get_nc()
    in_maps = []
    for c in range(N_CORES):
        in_maps.append(
            {
                "pc": pc_b[c * A_PER_CORE : (c + 1) * A_PER_CORE],
                "tcs": tcs[c * G_PER_CORE : (c + 1) * G_PER_CORE],
            }
        )
    res = run_bass_kernel_spmd(
        nc, in_maps, list(range(N_CORES)), trace=_TRACE, **_TRACE_KWARGS
    )
    out = np.concatenate([r["out"] for r in res.results]).astype(np.float32)
    if _TRACE:
        kernel.last_results = res
    return out


# revision 8
# speedup vs baseline: 2.3733x; 1.0026x over previous
"""Trainium2 Bass kernel for CorrectedPartialCharges.

out[i] = pc[i] + (total_charge[g] - seg_sum[g]) / n_atoms[g],  g = i // 256

Sharding: graphs are data-parallel across the 8 cores (4096 graphs /
1,048,576 atoms per core); segment sums and the gather-broadcast stay
device-local. On each core, partition p owns 32 contiguous graphs.

Wire format is bf16 (the 2e-2 rel-err budget allows it): node charges are
rounded to bf16 on the host, halving HBM traffic both ways; all device
accumulation is fp32. total_charge is pre-divided by 256 on the host so
the leftover is one fused scalar_tensor_tensor op.

Engine split (per [128, 2048] tile, k=8 graphs per partition):
  - Tensor: 4 accumulating identity matmuls fold each graph's 256 atoms
    into 64 PSUM columns (the bulk of the segment reduction, on an
    otherwise idle engine).
  - Vector: 1x reduce of the [128, k, 64] PSUM partials -> seg, fused
    leftover, and most per-graph tensor_scalar adds.
  - Scalar/ACT: per-graph bias adds for a subset of j-indices... no --
    whole-tile granularity (the tile dependency tracker serializes
    cross-engine writes to one tile), so ACT owns the adds of designated
    tiles and Vector the rest.
  - GpSimd: only the small identity + total-charge DMAs (SWDGE); its
    tensor ops have ~4.7us dispatch overhead and are useless here.
DMA: loads/stores alternate between the two HWDGE rings; the last store
is split across both rings to shorten the drain tail.
"""

import ml_dtypes
import numpy as np

import concourse.bacc as bacc
import concourse.bass as bass  # noqa: F401
import concourse.mybir as mybir
import concourse.tile as tile
from concourse.bass_utils import run_bass_kernel_spmd

N_CORES = 8
ATOMS_PER_GRAPH = 256
N_GRAPHS = 32768
N_ATOMS = N_GRAPHS * ATOMS_PER_GRAPH
P = 128

G_PER_CORE = N_GRAPHS // N_CORES          # 4096 graphs per core
A_PER_CORE = G_PER_CORE * ATOMS_PER_GRAPH  # 1,048,576 atoms per core

# Knobs read by test.py when experimenting.
NT = 4                    # tiles per core along the free dim
PSUM_W = 64               # columns per graph after the matmul pre-reduce
RED_MODE = "matmul"       # "matmul" | "halve"
TILE_ORDER = (2, 3, 0, 1)  # processing order; first listed loads first
# add engine per tile ("vector" or "scalar"); ACT tiles should be early in
# TILE_ORDER so the ACT chain starts as soon as possible.
TILE_ADD_ENGINE = {2: "scalar", 3: "vector", 0: "vector", 1: "vector"}
# within a vector tile, j-indices >= this go to ACT?  (kept simple: no)
SPLIT_LAST_STORE = True

_TRACE = False
_TRACE_KWARGS = {}


def _build(nt=None, order=None, add_eng=None, red_mode=None, psum_w=None,
           split_last=None):
    nt = NT if nt is None else nt
    order = TILE_ORDER if order is None else order
    add_eng = TILE_ADD_ENGINE if add_eng is None else add_eng
    red_mode = RED_MODE if red_mode is None else red_mode
    psum_w = PSUM_W if psum_w is None else psum_w
    split_last = SPLIT_LAST_STORE if split_last is None else split_last

    ap_free = A_PER_CORE // P     # 8192 atoms per partition
    gp = G_PER_CORE // P          # 32 graphs per partition
    w = ap_free // nt             # atoms per partition per tile
    k = w // ATOMS_PER_GRAPH      # graphs per partition per tile
    half = ATOMS_PER_GRAPH // 2
    n_pass = ATOMS_PER_GRAPH // psum_w
    assert ap_free % nt == 0 and w % ATOMS_PER_GRAPH == 0
    assert k * psum_w * 4 <= 2048, "psum accumulation group must fit one bank"
    assert tuple(sorted(order)) == tuple(range(nt))

    nc = bacc.Bacc(None, target_bir_lowering=False, enable_partition_id=False)

    pc = nc.dram_tensor("pc", [A_PER_CORE], mybir.dt.bfloat16, kind="ExternalInput")
    # total_charge / 256, fp32
    tcs = nc.dram_tensor("tcs", [G_PER_CORE], mybir.dt.float32, kind="ExternalInput")
    eye = nc.dram_tensor("eye", [P * P], mybir.dt.bfloat16, kind="ExternalInput")
    out = nc.dram_tensor("out", [A_PER_CORE], mybir.dt.bfloat16, kind="ExternalOutput")

    pc_v = pc[:].rearrange("(p n) -> p n", p=P)
    out_v = out[:].rearrange("(p n) -> p n", p=P)
    tcs_v = tcs[:].rearrange("(p k) -> p k", p=P)
    eye_v = eye[:].rearrange("(p n) -> p n", p=P)

    with tile.TileContext(nc) as tc:
        with (
            tc.tile_pool(name="io", bufs=nt) as io_pool,
            tc.tile_pool(name="half", bufs=2) as half_pool,
            tc.tile_pool(name="small", bufs=2 * nt) as small_pool,
            tc.tile_pool(name="consts", bufs=1) as const_pool,
            tc.tile_pool(name="psum", bufs=nt, space="PSUM") as psum_pool,
        ):
            # Queue every input tile load up front, alternating HWDGE rings.
            xs = {}
            for i, t in enumerate(order):
                x = io_pool.tile([P, w], mybir.dt.bfloat16, tag="x")
                eng = nc.sync if i % 2 == 0 else nc.scalar
                eng.dma_start(out=x[:], in_=pc_v[:, t * w : (t + 1) * w])
                xs[t] = x

            eye_tile = const_pool.tile([P, P], mybir.dt.bfloat16, tag="eye")
            nc.gpsimd.dma_start(out=eye_tile[:], in_=eye_v)
            tc_tile = const_pool.tile([P, gp], mybir.dt.float32, tag="tc")
            nc.gpsimd.dma_start(out=tc_tile[:], in_=tcs_v)

            for i, t in enumerate(order):
                x = xs[t]
                x3 = x[:].rearrange("p (k a) -> p k a", a=ATOMS_PER_GRAPH)

                seg = small_pool.tile([P, k], mybir.dt.float32, tag="seg")
                if red_mode == "matmul":
                    # Fold 256 atoms -> psum_w columns per graph with
                    # accumulating identity matmuls on the Tensor engine.
                    ps = psum_pool.tile([P, k, psum_w], mybir.dt.float32, tag="ps")
                    for s in range(n_pass):
                        nc.tensor.matmul(
                            ps[:],
                            eye_tile[:],
                            x3[:, :, s * psum_w : (s + 1) * psum_w],
                            start=(s == 0),
                            stop=(s == n_pass - 1),
                        )
                    nc.vector.reduce_sum(
                        out=seg[:], in_=ps[:], axis=mybir.AxisListType.X
                    )
                else:
                    u = half_pool.tile([P, k * half], mybir.dt.bfloat16, tag="u")
                    u3 = u[:].rearrange("p (k a) -> p k a", a=half)
                    nc.vector.tensor_add(
                        out=u3, in0=x3[:, :, 0:half],
                        in1=x3[:, :, half:ATOMS_PER_GRAPH],
                    )
                    nc.vector.reduce_sum(out=seg[:], in_=u3, axis=mybir.AxisListType.X)

                # left = (seg * -1/256) + tc/256   (fused)
                left = small_pool.tile([P, k], mybir.dt.float32, tag="left")
                nc.vector.scalar_tensor_tensor(
                    out=left[:],
                    in0=seg[:],
                    scalar=-1.0 / ATOMS_PER_GRAPH,
                    in1=tc_tile[:, t * k : (t + 1) * k],
                    op0=mybir.AluOpType.mult,
                    op1=mybir.AluOpType.add,
                )

                for j in range(k):
                    blk = x[:, j * ATOMS_PER_GRAPH : (j + 1) * ATOMS_PER_GRAPH]
                    if add_eng[t] == "scalar":
                        nc.scalar.add(out=blk, in_=blk, add=left[:, j : j + 1])
                    else:
                        nc.vector.tensor_scalar_add(
                            out=blk, in0=blk, scalar1=left[:, j : j + 1]
                        )
                if split_last and i == nt - 1:
                    hw_ = w // 2
                    nc.scalar.dma_start(
                        out=out_v[:, t * w : t * w + hw_], in_=x[:, 0:hw_]
                    )
                    nc.sync.dma_start(
                        out=out_v[:, t * w + hw_ : (t + 1) * w], in_=x[:, hw_:w]
                    )
                else:
                    # stores for ACT-add tiles go on sync so they don't queue
                    # behind the ACT add chain; vector tiles use the ACT ring.
                    eng = nc.sync if add_eng[t] == "scalar" else nc.scalar
                    eng.dma_start(out=out_v[:, t * w : (t + 1) * w], in_=x[:])

    nc.finalize()
    return nc


_NC_CACHE = {}


def _get_nc():
    key = (NT, TILE_ORDER, tuple(sorted(TILE_ADD_ENGINE.items())), RED_MODE,
           PSUM_W, SPLIT_LAST_STORE)
    if key not in _NC_CACHE:
        _NC_CACHE[key] = _build()
    return _NC_CACHE[key]


def _cpu_fallback(pc, total_charge, batch, n_atoms):
    num_segments = n_atoms.shape[0]
    seg = np.bincount(batch, weights=pc.astype(np.float64), minlength=num_segments)
    leftover = (total_charge - seg.astype(np.float32)) / n_atoms.astype(np.float32)
    return (pc + leftover[batch]).astype(np.float32)


_EYE = None


def kernel(**inputs) -> np.ndarray:
    global _EYE
    pc = np.ascontiguousarray(
        np.asarray(inputs["node_outputs"], dtype=np.float32).reshape(-1)
    )
    total_charge = np.ascontiguousarray(
        np.asarray(inputs["total_charge"], dtype=np.float32).reshape(-1)
    )
    batch = np.asarray(inputs["batch"]).reshape(-1)
    n_atoms = np.ascontiguousarray(np.asarray(inputs["n_atoms"], dtype=np.int32).reshape(-1))

    # The device kernel hardcodes the uniform 256-atoms-per-graph layout the
    # reference generator produces; anything else goes through numpy.
    if (
        pc.shape[0] != N_ATOMS
        or total_charge.shape[0] != N_GRAPHS
        or not np.all(n_atoms == ATOMS_PER_GRAPH)
        or not np.array_equal(
            batch.astype(np.int64),
            np.arange(N_ATOMS, dtype=np.int64) // ATOMS_PER_GRAPH,
        )
    ):
        return _cpu_fallback(pc, total_charge, batch, n_atoms)

    pc_b = pc.astype(ml_dtypes.bfloat16)
    tcs = (total_charge * (1.0 / ATOMS_PER_GRAPH)).astype(np.float32)
    if _EYE is None:
        _EYE = np.eye(P, dtype=ml_dtypes.bfloat16).reshape(-1)

    nc = _get_nc()
    in_maps = []
    for c in range(N_CORES):
        in_maps.append(
            {
                "pc": pc_b[c * A_PER_CORE : (c + 1) * A_PER_CORE],
                "tcs": tcs[c * G_PER_CORE : (c + 1) * G_PER_CORE],
                "eye": _EYE,
            }
        )
    res = run_bass_kernel_spmd(
        nc, in_maps, list(range(N_CORES)), trace=_TRACE, **_TRACE_KWARGS
    )
    out = np.concatenate([r["out"] for r in res.results]).astype(np.float32)
    if _TRACE:
        kernel.last_results = res
    return out
